# revision 27
# baseline (speedup 1.0000x reference)
"""Trainium2 Bass kernel for gated multi-head attention (B=8, N=1568, C=768, H=12).

Sharding: data-parallel over batch — core b computes batch element b entirely
locally (weights replicated), host gathers. Feature-major layouts ([channels,
tokens]) so every matmul contracts on partitions.

Fused single-pipeline design: the attention steady state is paced by ACT's
exp stream (one [kw, 2x392] EXP per (chunk, kt) step, ~0.95us each). All other
work rides in the PE/DVE slack of that stream:

  prologue   pair-0 QKV/gate/vt/zq (the cold-clock HAM warmup IS this real work)
  steady     24 chunks = (pair, qt) x 13 kt steps:
               scores (2 MM, zero-padded K=128 trick) -> EXP -> AV (lag 1 step)
               + interleaved filler: pairs 1-5 QKV chains, gates, V transposes,
                 zq copies; per-pair normalization (denominator recip +
                 basis-broadcast matmuls)
  tail       output projection (24 chains) + DMA out

PSUM budget: score slots 2x[128,2,512] (4 banks) + AV accumulators bufs=3
(2 active + 1 draining, [65,392] each) + 1 filler bank (QKV/gate/transpose/
proj chains, one at a time) = 8 banks.

Gate: sigmoid via tanh (same ACT table set as exp); Qg' = (tanh+1)*Q =
2*sigmoid*Q, the 2x per side folded into the exp scale (SCALE/4). Softmax
needs no max-subtraction: |scores*scale| < 1 for this data. Denominator rides
in AV row 64 via a ones-column in vt; normalization multiplies by a
basis-matmul broadcast of the reciprocal.
"""

import numpy as np
import ml_dtypes
from contextlib import ExitStack

import concourse.bass as bass
import concourse.tile as tile
from concourse import bacc, mybir
from concourse.bass_utils import run_bass_kernel_spmd

BF16 = mybir.dt.bfloat16
F32 = mybir.dt.float32
AF = mybir.ActivationFunctionType
ALU = mybir.AluOpType

N_CORES = 8
N, C, H, HD = 1568, 768, 12, 64
SCALE = HD ** -0.5
QT = 392            # token free-dim tile (4 per pair)
KTS = [128] * 12 + [32]   # token partition tiles
NKT = len(KTS)
NCH = 24            # chunks = (pair, qt)


def build_nc():
    nc = bacc.Bacc(
        "TRN2",
        target_bir_lowering=False,
        debug=False,
        enable_asserts=False,
        num_devices=N_CORES,
    )
    xt = nc.dram_tensor("xt", [C, N], BF16, kind="ExternalInput").ap()
    qkv_wt = nc.dram_tensor("qkv_wt", [C, 3 * C], BF16, kind="ExternalInput").ap()
    gwq = nc.dram_tensor("gwq", [128, 128], BF16, kind="ExternalInput").ap()
    gwk = nc.dram_tensor("gwk", [128, 128], BF16, kind="ExternalInput").ap()
    gwv = nc.dram_tensor("gwv", [128, 128], BF16, kind="ExternalInput").ap()
    gb = nc.dram_tensor("gb", [128, 1], F32, kind="ExternalInput").ap()
    proj_wt = nc.dram_tensor("proj_wt", [C, C], BF16, kind="ExternalInput").ap()
    proj_b = nc.dram_tensor("proj_b", [C, 1], F32, kind="ExternalInput").ap()
    ident = nc.dram_tensor("ident", [128, 128], BF16, kind="ExternalInput").ap()
    out = nc.dram_tensor("out", [C, N], F32, kind="ExternalOutput").ap()

    with tile.TileContext(nc) as tc, ExitStack() as ES:
        constP = ES.enter_context(tc.tile_pool(name="const", bufs=1))
        kgP = ES.enter_context(tc.tile_pool(name="kgsb", bufs=1))
        qvP = ES.enter_context(tc.tile_pool(name="qvsb", bufs=3))
        vtP = ES.enter_context(tc.tile_pool(name="vtsb", bufs=1))
        attnP = ES.enter_context(tc.tile_pool(name="attnsb", bufs=1))
        zqP = ES.enter_context(tc.tile_pool(name="zqp", bufs=1))
        eP = ES.enter_context(tc.tile_pool(name="esb", bufs=5))
        oP = ES.enter_context(tc.tile_pool(name="osb", bufs=4))
        # PSUM pools are opened in phase scopes below; units reach the current
        # filler pool via PS["f"].
        PS = {}

        # ---------------- DMA in (priority order) ----------------
        xt_sb = []
        qw_sb = []
        for k in range(6):
            xt_sb.append(constP.tile([128, N], BF16, tag=f"xt{k}", name=f"xt{k}"))
            nc.sync.dma_start(xt_sb[k][:, :], xt[k * 128:(k + 1) * 128, :])
            qw_sb.append(constP.tile([128, 3 * C], BF16, tag=f"qw{k}", name=f"qw{k}"))
            nc.sync.dma_start(qw_sb[k][:, :], qkv_wt[k * 128:(k + 1) * 128, :])
        ident_sb = constP.tile([128, 128], BF16, tag="ident", name="ident")
        nc.sync.dma_start(ident_sb[:, :], ident)
        gw_sb = {}
        for nm, t in (("gwq", gwq), ("gwk", gwk), ("gwv", gwv)):
            gw_sb[nm] = constP.tile([128, 128], BF16, tag=nm, name=nm)
            nc.sync.dma_start(gw_sb[nm][:, :], t)
        gb_sb = constP.tile([128, 1], F32, tag="gb", name="gb")
        nc.sync.dma_start(gb_sb[:, :], gb)
        pw_sb = []
        pb_sb = []
        for k in range(6):
            pw_sb.append(constP.tile([128, C], BF16, tag=f"pw{k}", name=f"pw{k}"))
            nc.sync.dma_start(pw_sb[k][:, :], proj_wt[k * 128:(k + 1) * 128, :])
            pb_sb.append(constP.tile([128, 1], F32, tag=f"pb{k}", name=f"pb{k}"))
            nc.sync.dma_start(pb_sb[k][:, :], proj_b[k * 128:(k + 1) * 128, :])

        # ---------------- static SBUF ----------------
        # basis[qt]: one-hot row 32*qt for the reciprocal broadcast matmul
        basis_sb = []
        for qt in range(4):
            bt = constP.tile([128, 64], F32, tag=f"basis{qt}", name=f"basis{qt}")
            nc.any.memset(bt[:, :], 0.0)
            nc.any.memset(bt[32 * qt:32 * qt + 1, :], 1.0)
            basis_sb.append(bt)

        kg_sb = [kgP.tile([128, N], BF16, tag=f"kg{p}", name=f"kg{p}")
                 for p in range(6)]
        attn_sb = [attnP.tile([128, N], BF16, tag=f"a{p}", name=f"a{p}")
                   for p in range(6)]
        den_sb = attnP.tile([128, H * QT], F32, tag="den", name="den")
        rec_sb = attnP.tile([128, H * QT], F32, tag="recip", name="recip")
        nc.any.memset(den_sb[:, :], 1.0)
        nc.any.memset(rec_sb[:, :], 0.0)

        # per-(pair, kt) V^T tiles: [kw, 2 heads x (64 d + ones-col)] — separate
        # tiles per pair so pair p+1's transposes have no WAR on pair p's AV
        # reads
        vt_sb = {}
        for p in range(6):
            for kt in range(NKT):
                t = vtP.tile([128, 130], BF16, tag=f"vt{p}_{kt}",
                             name=f"vt{p}_{kt}")
                ones_col = t[:, :].rearrange("p (h e) -> p h e", e=65)[:, :, 64]
                nc.any.memset(ones_col, 1.0)
                vt_sb[p, kt] = t

        # zero-padded per-head Q, two alternating persistent sets
        zq_sb = {}
        for s in range(2):
            for hh in range(2):
                off = hh * 64
                z = zqP.tile([128, N], BF16, tag=f"zq{s}{hh}", name=f"zq{s}{hh}")
                nc.gpsimd.memset(z[64 - off:128 - off, :], 0.0)
                zq_sb[s, hh] = z

        # ---------------- filler work units ----------------
        # Each unit emits a small PE chain (+ its drain) through the single
        # filler PSUM bank. Units are generators of nothing; emission order is
        # the schedule.
        q_tiles = {}
        v_tiles = {}

        def unit_qkv(p, which, qt):
            # which: 0=Q(m=p),1=K(m=6+p),2=V(m=12+p); one qt strip
            m = p + 6 * which
            sl = slice(qt * QT, (qt + 1) * QT)
            if which == 0:
                if p not in q_tiles:
                    q_tiles[p] = qvP.tile([128, N], BF16, tag="q", name=f"q{p}")
                dst = q_tiles[p]
            elif which == 1:
                dst = kg_sb[p]
            else:
                if p not in v_tiles:
                    v_tiles[p] = qvP.tile([128, N], BF16, tag="v", name=f"v{p}")
                dst = v_tiles[p]
            ps = PS["f"].tile([128, QT], F32, tag="f", name="f")
            for k in range(6):
                nc.tensor.matmul(ps[:, :],
                                 lhsT=qw_sb[k][:, m * 128:(m + 1) * 128],
                                 rhs=xt_sb[k][:, sl],
                                 start=(k == 0), stop=(k == 5))
            nc.vector.tensor_copy(dst[:, sl], ps[:, :])

        def unit_gate(p, qt):
            sl = slice(qt * QT, (qt + 1) * QT)
            g = eP.tile([128, QT], BF16, tag="g", name="g")
            ps = PS["f"].tile([128, QT], F32, tag="f", name="f")
            nc.tensor.matmul(ps[:, :], lhsT=gw_sb["gwq"][:, :],
                             rhs=q_tiles[p][:, sl], start=True, stop=False)
            nc.tensor.matmul(ps[:, :], lhsT=gw_sb["gwk"][:, :],
                             rhs=kg_sb[p][:, sl], start=False, stop=False)
            nc.tensor.matmul(ps[:, :], lhsT=gw_sb["gwv"][:, :],
                             rhs=v_tiles[p][:, sl], start=False, stop=True)
            nc.scalar.activation(g[:, :], ps[:, :], AF.Tanh,
                                 bias=gb_sb[:, 0:1], scale=0.5)
            # Qg' = (t+1)*Q in place; Kg' = (t+1)*K in place
            nc.vector.scalar_tensor_tensor(
                q_tiles[p][:, sl], g[:, :], 1.0, q_tiles[p][:, sl],
                op0=ALU.add, op1=ALU.mult)
            nc.vector.scalar_tensor_tensor(
                kg_sb[p][:, sl], g[:, :], 1.0, kg_sb[p][:, sl],
                op0=ALU.add, op1=ALU.mult)

        def unit_zq(p, hh, qt):
            off = hh * 64
            sl = slice(qt * QT, (qt + 1) * QT)
            nc.vector.tensor_copy(zq_sb[p % 2, hh][off:off + 64, sl],
                                  q_tiles[p][off:off + 64, sl])

        def unit_vt(p, kt):
            kw = KTS[kt]
            vsrc = v_tiles[p][:, kt * 128:kt * 128 + kw]
            ps = PS["f"].tile([128, 128], BF16, tag="f", name="f")
            nc.tensor.transpose(ps[0:kw, 0:128], vsrc, ident_sb[:, :])
            dst = vt_sb[p, kt][0:kw, :].rearrange(
                "p (h e) -> p h e", e=65)[:, :, 0:64]
            nc.vector.tensor_copy(
                dst, ps[0:kw, :].rearrange("p (h e) -> p h e", e=64))

        def unit_proj(m, qt):
            sl = slice(qt * QT, (qt + 1) * QT)
            ps = PS["f"].tile([128, QT], F32, tag="f", name="f")
            for k in range(6):
                nc.tensor.matmul(ps[:, :],
                                 lhsT=pw_sb[k][:, m * 128:(m + 1) * 128],
                                 rhs=attn_sb[k][:, sl],
                                 start=(k == 0), stop=(k == 5))
            o = oP.tile([128, QT], F32, tag="o", name="o")
            nc.scalar.activation(o[:, :], ps[:, :], AF.Identity,
                                 bias=pb_sb[m][:, 0:1])
            nc.sync.dma_start(out[m * 128:(m + 1) * 128, sl], o[:, :])

        def strip_units(p, qt):
            # all prep for one qt strip of a pair: K,V,Q halves + gate + zq
            u = []
            for which in (1, 2, 0):   # K first (scores lhsT), V, Q
                u.append(lambda p=p, w=which, qt=qt: unit_qkv(p, w, qt))
            u.append(lambda p=p, qt=qt: unit_gate(p, qt))
            u.append(lambda p=p, qt=qt: unit_zq(p, 0, qt))
            u.append(lambda p=p, qt=qt: unit_zq(p, 1, qt))
            return u

        # ---------------- attention pipeline ----------------
        e_tiles = {}
        avps = {}

        def emit_scores(ci, kt):
            p, qt = divmod(ci, 4)
            kw = KTS[kt]
            kb = kt * 128
            sps = PS["s"].tile([128, 2, 512], F32, tag="s", name="s")
            for hh in range(2):
                nc.tensor.matmul(
                    sps[0:kw, hh, 0:QT],
                    lhsT=kg_sb[p][:, kb:kb + kw],
                    rhs=zq_sb[p % 2, hh][:, qt * QT:(qt + 1) * QT],
                    start=True, stop=True,
                )
            e = eP.tile([128, 2 * QT], BF16, tag="e", name="e")
            nc.scalar.activation(
                e[0:kw, :].rearrange("p (s n) -> p s n", s=2),
                sps[0:kw, :, 0:QT], AF.Exp, scale=SCALE / 4.0,
            )
            e_tiles[ci, kt] = e

        def emit_av(ci, kt):
            p, qt = divmod(ci, 4)
            kw = KTS[kt]
            if kt == 0:
                for hh in range(2):
                    avps[ci, hh] = PS["av"].tile([65, QT], F32, tag="av",
                                                 name="av")
            e = e_tiles[ci, kt]
            for hh in range(2):
                nc.tensor.matmul(
                    avps[ci, hh][:, :],
                    lhsT=vt_sb[p, kt][0:kw, hh * 65:hh * 65 + 65],
                    rhs=e[0:kw, hh * QT:(hh + 1) * QT],
                    start=(kt == 0), stop=(kt == NKT - 1),
                    skip_group_check=True,
                )
            if kt == NKT - 1:
                del e_tiles[ci, kt]

        def emit_drain(ci):
            p, qt = divmod(ci, 4)
            for hh in range(2):
                h = 2 * p + hh
                off = hh * 64
                nc.vector.tensor_copy(
                    attn_sb[p][off:off + 64, qt * QT:(qt + 1) * QT],
                    avps[ci, hh][0:64, :])
                nc.vector.tensor_copy(
                    den_sb[32 * qt:32 * qt + 1, h * QT:(h + 1) * QT],
                    avps[ci, hh][64:65, :])

        def emit_norm(p):
            # after the pair's last drain: reciprocal + broadcast + multiply.
            # bc borrows AV slots (not score slots) so the next pair's score
            # stream — the ACT pacer's input — is never blocked behind norm.
            for hh in range(2):
                h = 2 * p + hh
                off = hh * 64
                nc.vector.reciprocal_approx_fast(
                    rec_sb[:, h * QT:(h + 1) * QT],
                    den_sb[:, h * QT:(h + 1) * QT])
                for qt in range(4):
                    bc = PS["av"].tile([64, QT], F32, tag="av", name="av")
                    nc.tensor.matmul(
                        bc[:, :], lhsT=basis_sb[qt][:, :],
                        rhs=rec_sb[:, h * QT:(h + 1) * QT],
                        start=True, stop=True)
                    nc.vector.tensor_mul(
                        attn_sb[p][off:off + 64, qt * QT:(qt + 1) * QT],
                        attn_sb[p][off:off + 64, qt * QT:(qt + 1) * QT],
                        bc[:, :])

        # prologue: dependency-free dummy matmuls keep the HAM clock-gate warm
        # while the input DMAs land, then pair-0 strip 0 + its vt tiles; the
        # rest of pair 0 is front-loaded filler so the EXP stream starts early
        with tc.tile_pool(name="ps_pro", bufs=6, space="PSUM") as psPro, \
             tc.tile_pool(name="ps_w", bufs=1, space="PSUM") as psW:
            PS["f"] = psPro
            wrm = constP.tile([128, 512], BF16, tag="wrm", name="wrm")
            nc.any.memset(wrm[:, :], 0.0)
            wps = psW.tile([128, 512], F32, tag="w", name="w")
            for i in range(26):
                nc.tensor.matmul(wps[:, :], lhsT=wrm[:, 0:128], rhs=wrm[:, :],
                                 start=(i == 0), stop=(i == 25))
            for u in strip_units(0, 0):
                u()
            for kt in range(3):
                unit_vt(0, kt)

        # steady pipeline pools: scores 2x2 banks, AV 3, filler 1
        with tc.tile_pool(name="ps_s", bufs=2, space="PSUM") as psS, \
             tc.tile_pool(name="ps_av", bufs=3, space="PSUM") as psAV, \
             tc.tile_pool(name="ps_f", bufs=1, space="PSUM") as psF:
            PS["s"], PS["av"], PS["f"] = psS, psAV, psF

            # filler queue: (deadline_step, unit). Pair-0 remnants front-
            # loaded to feed the first chunk's kt progression; pairs 1-5
            # spread over the previous pair's 52-step window.
            filler = []
            strip_dl = {0: 0, 1: 2, 2: 5, 3: 8}
            for qt in (1, 2, 3):
                for u in strip_units(0, qt):
                    filler.append((strip_dl[qt], u))
            for kt in range(3, NKT):
                # vt(0,kt) reads V tokens [128kt, 128kt+kw): must follow the
                # strip that covers its last token, and be ready by g=kt+1
                s_last = (128 * kt + KTS[kt] - 1) // QT
                filler.append((strip_dl[s_last] + 1,
                               lambda kt=kt: unit_vt(0, kt)))
            filler.sort(key=lambda t: t[0])
            kts_by_strip = {}
            for kt in range(NKT):
                kts_by_strip.setdefault((128 * kt + KTS[kt] - 1) // QT,
                                        []).append(kt)
            for p in range(1, 6):
                pu = []
                for qt in range(4):
                    pu.extend(strip_units(p, qt))
                    for kt in kts_by_strip[qt]:
                        pu.append(lambda p=p, kt=kt: unit_vt(p, kt))
                # pair 1's window starts after pair-0 remnants are due
                w0 = 10 if p == 1 else 52 * (p - 1)
                wlen = 52 * p - w0
                npu = len(pu)
                for i, u in enumerate(pu):
                    filler.append((w0 + int((i + 1) * wlen / npu), u))
            fill_i = 0

            LAG = 1
            for g in range(NCH * NKT + LAG):
                ci, kt = divmod(g, NKT)
                if ci < NCH:
                    emit_scores(ci, kt)
                s = g - LAG
                if s >= 0 and s < NCH * NKT:
                    c2, j = divmod(s, NKT)
                    emit_av(c2, j)
                    if j == NKT - 1:
                        emit_drain(c2)
                        p2, qt2 = divmod(c2, 4)
                        if qt2 == 3:
                            emit_norm(p2)
                while fill_i < len(filler) and filler[fill_i][0] <= g:
                    filler[fill_i][1]()
                    fill_i += 1
            while fill_i < len(filler):
                filler[fill_i][1]()
                fill_i += 1

        # tail: output projection, deep psum pipeline
        with tc.tile_pool(name="ps_t", bufs=6, space="PSUM") as psT:
            PS["f"] = psT
            for m in range(6):
                for qt in range(4):
                    unit_proj(m, qt)

    nc.compile()
    return nc


_CACHE = {}


def _get_nc():
    if "nc" not in _CACHE:
        _CACHE["nc"] = build_nc()
    return _CACHE["nc"]


def make_in_maps(x, qkv_w, pgate_w, pgate_b, proj_w, proj_b):
    bf = ml_dtypes.bfloat16
    x = np.asarray(x, np.float32)
    qkv_w = np.asarray(qkv_w, np.float32)
    pgate_w = np.asarray(pgate_w, np.float32)
    pgate_b = np.asarray(pgate_b, np.float32)
    proj_w = np.asarray(proj_w, np.float32)
    proj_b = np.asarray(proj_b, np.float32)

    common = {
        "qkv_wt": np.ascontiguousarray(qkv_w.T).astype(bf),
        "proj_wt": np.ascontiguousarray(proj_w.T).astype(bf),
        "proj_b": np.ascontiguousarray(proj_b.reshape(C, 1)),
        "ident": np.eye(128, dtype=np.float32).astype(bf),
        # gate bias folded for tanh form: tanh(0.5*pre + 0.5*b)
        "gb": np.concatenate([pgate_b, pgate_b]).reshape(128, 1).astype(np.float32) * 0.5,
    }
    for nm, sl in (("gwq", slice(0, 64)), ("gwk", slice(64, 128)),
                   ("gwv", slice(128, 192))):
        w = pgate_w[:, sl].T  # [d, e] = lhsT
        bd = np.zeros((128, 128), np.float32)
        bd[0:64, 0:64] = w
        bd[64:128, 64:128] = w
        common[nm] = bd.astype(bf)

    return [
        {**common, "xt": np.ascontiguousarray(x[b].T).astype(bf)}
        for b in range(N_CORES)
    ]


def kernel(x, qkv_w, pgate_w, pgate_b, proj_w, proj_b, num_frames=None, **_unused):
    in_maps = make_in_maps(x, qkv_w, pgate_w, pgate_b, proj_w, proj_b)
    nc = _get_nc()
    res = run_bass_kernel_spmd(nc, in_maps, core_ids=list(range(N_CORES)))
    out = np.stack([np.asarray(res.results[b]["out"], np.float32).T
                    for b in range(N_CORES)])
    return np.ascontiguousarray(out)


# revision 33
# speedup vs baseline: 1.0395x; 1.0395x over previous
"""Trainium2 Bass kernel for gated multi-head attention (B=8, N=1568, C=768, H=12).

Sharding: data-parallel over batch — core b computes batch element b entirely
locally (weights replicated), host gathers. Feature-major layouts ([channels,
tokens]) so every matmul contracts on partitions.

Fused single-pipeline design: the attention steady state is paced by ACT's
exp stream (one [kw, 2x392] EXP per (chunk, kt) step, ~0.95us each). All other
work rides in the PE/DVE slack of that stream:

  prologue   pair-0 QKV/gate/vt/zq (the cold-clock HAM warmup IS this real work)
  steady     24 chunks = (pair, qt) x 13 kt steps:
               scores (2 MM, zero-padded K=128 trick) -> EXP -> AV (lag 1 step)
               + interleaved filler: pairs 1-5 QKV chains, gates, V transposes,
                 zq copies; per-pair normalization (denominator recip +
                 basis-broadcast matmuls)
  tail       output projection (24 chains) + DMA out

PSUM budget: score slots 2x[128,2,512] (4 banks) + AV accumulators bufs=3
(2 active + 1 draining, [65,392] each) + 1 filler bank (QKV/gate/transpose/
proj chains, one at a time) = 8 banks.

Gate: sigmoid via tanh (same ACT table set as exp); Qg' = (tanh+1)*Q =
2*sigmoid*Q, the 2x per side folded into the exp scale (SCALE/4). Softmax
needs no max-subtraction: |scores*scale| < 1 for this data. Denominator rides
in AV row 64 via a ones-column in vt; normalization multiplies by a
basis-matmul broadcast of the reciprocal.
"""

import numpy as np
import ml_dtypes
from contextlib import ExitStack

import concourse.bass as bass
import concourse.tile as tile
from concourse import bacc, mybir
from concourse.bass_utils import run_bass_kernel_spmd

BF16 = mybir.dt.bfloat16
F32 = mybir.dt.float32
AF = mybir.ActivationFunctionType
ALU = mybir.AluOpType

N_CORES = 8
N, C, H, HD = 1568, 768, 12, 64
SCALE = HD ** -0.5
QT = 392            # token free-dim tile (4 per pair)
KTS = [128] * 12 + [32]   # token partition tiles
NKT = len(KTS)
NCH = 24            # chunks = (pair, qt)


def build_nc():
    nc = bacc.Bacc(
        "TRN2",
        target_bir_lowering=False,
        debug=False,
        enable_asserts=False,
        num_devices=N_CORES,
    )
    xt = nc.dram_tensor("xt", [C, N], BF16, kind="ExternalInput").ap()
    qkv_wt = nc.dram_tensor("qkv_wt", [C, 3 * C], BF16, kind="ExternalInput").ap()
    gwq = nc.dram_tensor("gwq", [128, 128], BF16, kind="ExternalInput").ap()
    gwk = nc.dram_tensor("gwk", [128, 128], BF16, kind="ExternalInput").ap()
    gwv = nc.dram_tensor("gwv", [128, 128], BF16, kind="ExternalInput").ap()
    gb = nc.dram_tensor("gb", [128, 1], F32, kind="ExternalInput").ap()
    proj_wt = nc.dram_tensor("proj_wt", [C, C], BF16, kind="ExternalInput").ap()
    proj_b = nc.dram_tensor("proj_b", [C, 1], F32, kind="ExternalInput").ap()
    ident = nc.dram_tensor("ident", [128, 128], BF16, kind="ExternalInput").ap()
    out = nc.dram_tensor("out", [C, N], F32, kind="ExternalOutput").ap()

    with tile.TileContext(nc) as tc, ExitStack() as ES:
        constP = ES.enter_context(tc.tile_pool(name="const", bufs=1))
        kgP = ES.enter_context(tc.tile_pool(name="kgsb", bufs=1))
        qvP = ES.enter_context(tc.tile_pool(name="qvsb", bufs=3))
        vtP = ES.enter_context(tc.tile_pool(name="vtsb", bufs=1))
        attnP = ES.enter_context(tc.tile_pool(name="attnsb", bufs=1))
        zqP = ES.enter_context(tc.tile_pool(name="zqp", bufs=1))
        eP = ES.enter_context(tc.tile_pool(name="esb", bufs=5))
        oP = ES.enter_context(tc.tile_pool(name="osb", bufs=4))
        # PSUM pools are opened in phase scopes below; units reach the current
        # filler pool via PS["f"].
        PS = {}

        # ---------------- DMA in (priority order) ----------------
        xt_sb = []
        qw_sb = []
        for k in range(6):
            xt_sb.append(constP.tile([128, N], BF16, tag=f"xt{k}", name=f"xt{k}"))
            nc.sync.dma_start(xt_sb[k][:, :], xt[k * 128:(k + 1) * 128, :])
            qw_sb.append(constP.tile([128, 3 * C], BF16, tag=f"qw{k}", name=f"qw{k}"))
            nc.sync.dma_start(qw_sb[k][:, :], qkv_wt[k * 128:(k + 1) * 128, :])
        ident_sb = constP.tile([128, 128], BF16, tag="ident", name="ident")
        nc.sync.dma_start(ident_sb[:, :], ident)
        gw_sb = {}
        for nm, t in (("gwq", gwq), ("gwk", gwk), ("gwv", gwv)):
            gw_sb[nm] = constP.tile([128, 128], BF16, tag=nm, name=nm)
            nc.sync.dma_start(gw_sb[nm][:, :], t)
        gb_sb = constP.tile([128, 1], F32, tag="gb", name="gb")
        nc.sync.dma_start(gb_sb[:, :], gb)
        pw_sb = []
        pb_sb = []
        for k in range(6):
            pw_sb.append(constP.tile([128, C], BF16, tag=f"pw{k}", name=f"pw{k}"))
            nc.sync.dma_start(pw_sb[k][:, :], proj_wt[k * 128:(k + 1) * 128, :])
            pb_sb.append(constP.tile([128, 1], F32, tag=f"pb{k}", name=f"pb{k}"))
            nc.sync.dma_start(pb_sb[k][:, :], proj_b[k * 128:(k + 1) * 128, :])

        # ---------------- static SBUF ----------------
        # basis[qt]: one-hot row 32*qt for the reciprocal broadcast matmul
        basis_sb = []
        for qt in range(4):
            bt = constP.tile([128, 64], BF16, tag=f"basis{qt}", name=f"basis{qt}")
            nc.any.memset(bt[:, :], 0.0)
            nc.any.memset(bt[32 * qt:32 * qt + 1, :], 1.0)
            basis_sb.append(bt)

        kg_sb = [kgP.tile([128, N], BF16, tag=f"kg{p}", name=f"kg{p}")
                 for p in range(6)]
        attn_sb = [attnP.tile([128, N], BF16, tag=f"a{p}", name=f"a{p}")
                   for p in range(6)]
        den_sb = attnP.tile([128, H * QT], F32, tag="den", name="den")
        rec_sb = attnP.tile([128, H * QT], BF16, tag="recip", name="recip")
        rscr = attnP.tile([128, QT], F32, tag="rscr", name="rscr")
        nc.any.memset(den_sb[:, :], 1.0)
        nc.any.memset(rec_sb[:, :], 0.0)

        # per-(pair, kt) V^T tiles: [kw, 2 heads x (64 d + ones-col)] — separate
        # tiles per pair so pair p+1's transposes have no WAR on pair p's AV
        # reads
        vt_sb = {}
        for p in range(6):
            for kt in range(NKT):
                t = vtP.tile([128, 130], BF16, tag=f"vt{p}_{kt}",
                             name=f"vt{p}_{kt}")
                ones_col = t[:, :].rearrange("p (h e) -> p h e", e=65)[:, :, 64]
                nc.any.memset(ones_col, 1.0)
                vt_sb[p, kt] = t

        # zero-padded per-head Q, two alternating persistent sets
        zq_sb = {}
        for s in range(2):
            for hh in range(2):
                off = hh * 64
                z = zqP.tile([128, N], BF16, tag=f"zq{s}{hh}", name=f"zq{s}{hh}")
                nc.gpsimd.memset(z[64 - off:128 - off, :], 0.0)
                zq_sb[s, hh] = z

        # ---------------- filler work units ----------------
        # Each unit emits a small PE chain (+ its drain) through the single
        # filler PSUM bank. Units are generators of nothing; emission order is
        # the schedule.
        q_tiles = {}
        v_tiles = {}

        def unit_qkv(p, which, qt):
            # which: 0=Q(m=p),1=K(m=6+p),2=V(m=12+p); one qt strip
            m = p + 6 * which
            sl = slice(qt * QT, (qt + 1) * QT)
            if which == 0:
                if p not in q_tiles:
                    q_tiles[p] = qvP.tile([128, N], BF16, tag="q", name=f"q{p}")
                dst = q_tiles[p]
            elif which == 1:
                dst = kg_sb[p]
            else:
                if p not in v_tiles:
                    v_tiles[p] = qvP.tile([128, N], BF16, tag="v", name=f"v{p}")
                dst = v_tiles[p]
            ps = PS["f"].tile([128, QT], F32, tag="f", name="f")
            for k in range(6):
                nc.tensor.matmul(ps[:, :],
                                 lhsT=qw_sb[k][:, m * 128:(m + 1) * 128],
                                 rhs=xt_sb[k][:, sl],
                                 start=(k == 0), stop=(k == 5))
            nc.vector.tensor_copy(dst[:, sl], ps[:, :])

        def unit_gate(p, qt):
            sl = slice(qt * QT, (qt + 1) * QT)
            g = eP.tile([128, QT], BF16, tag="g", name="g")
            ps = PS["f"].tile([128, QT], F32, tag="f", name="f")
            nc.tensor.matmul(ps[:, :], lhsT=gw_sb["gwq"][:, :],
                             rhs=q_tiles[p][:, sl], start=True, stop=False)
            nc.tensor.matmul(ps[:, :], lhsT=gw_sb["gwk"][:, :],
                             rhs=kg_sb[p][:, sl], start=False, stop=False)
            nc.tensor.matmul(ps[:, :], lhsT=gw_sb["gwv"][:, :],
                             rhs=v_tiles[p][:, sl], start=False, stop=True)
            nc.scalar.activation(g[:, :], ps[:, :], AF.Tanh,
                                 bias=gb_sb[:, 0:1], scale=0.5)
            # Qg' = (t+1)*Q in place; Kg' = (t+1)*K in place
            nc.vector.scalar_tensor_tensor(
                q_tiles[p][:, sl], g[:, :], 1.0, q_tiles[p][:, sl],
                op0=ALU.add, op1=ALU.mult)
            nc.vector.scalar_tensor_tensor(
                kg_sb[p][:, sl], g[:, :], 1.0, kg_sb[p][:, sl],
                op0=ALU.add, op1=ALU.mult)

        def unit_zq(p, hh, qt):
            off = hh * 64
            sl = slice(qt * QT, (qt + 1) * QT)
            nc.vector.tensor_copy(zq_sb[p % 2, hh][off:off + 64, sl],
                                  q_tiles[p][off:off + 64, sl])

        def unit_vt(p, kt):
            kw = KTS[kt]
            vsrc = v_tiles[p][:, kt * 128:kt * 128 + kw]
            ps = PS["f"].tile([128, 128], BF16, tag="f", name="f")
            nc.tensor.transpose(ps[0:kw, 0:128], vsrc, ident_sb[:, :])
            dst = vt_sb[p, kt][0:kw, :].rearrange(
                "p (h e) -> p h e", e=65)[:, :, 0:64]
            nc.vector.tensor_copy(
                dst, ps[0:kw, :].rearrange("p (h e) -> p h e", e=64))

        def unit_proj(m, qt):
            sl = slice(qt * QT, (qt + 1) * QT)
            ps = PS["f"].tile([128, QT], F32, tag="f", name="f")
            for k in range(6):
                nc.tensor.matmul(ps[:, :],
                                 lhsT=pw_sb[k][:, m * 128:(m + 1) * 128],
                                 rhs=attn_sb[k][:, sl],
                                 start=(k == 0), stop=(k == 5))
            o = oP.tile([128, QT], F32, tag="o", name="o")
            nc.scalar.activation(o[:, :], ps[:, :], AF.Identity,
                                 bias=pb_sb[m][:, 0:1])
            nc.sync.dma_start(out[m * 128:(m + 1) * 128, sl], o[:, :])

        def strip_units(p, qt):
            # all prep for one qt strip of a pair: K,V,Q halves + gate + zq
            u = []
            for which in (1, 2, 0):   # K first (scores lhsT), V, Q
                u.append(lambda p=p, w=which, qt=qt: unit_qkv(p, w, qt))
            u.append(lambda p=p, qt=qt: unit_gate(p, qt))
            u.append(lambda p=p, qt=qt: unit_zq(p, 0, qt))
            u.append(lambda p=p, qt=qt: unit_zq(p, 1, qt))
            return u

        # ---------------- attention pipeline ----------------
        e_tiles = {}
        avps = {}

        def emit_scores(ci, kt):
            p, qt = divmod(ci, 4)
            kw = KTS[kt]
            kb = kt * 128
            sps = PS["s"].tile([128, 2, 512], F32, tag="s", name="s")
            for hh in range(2):
                nc.tensor.matmul(
                    sps[0:kw, hh, 0:QT],
                    lhsT=kg_sb[p][:, kb:kb + kw],
                    rhs=zq_sb[p % 2, hh][:, qt * QT:(qt + 1) * QT],
                    start=True, stop=True,
                )
            e = eP.tile([128, 2 * QT], BF16, tag="e", name="e")
            nc.scalar.activation(
                e[0:kw, :].rearrange("p (s n) -> p s n", s=2),
                sps[0:kw, :, 0:QT], AF.Exp, scale=SCALE / 4.0,
            )
            e_tiles[ci, kt] = e

        def emit_av(ci, kt):
            p, qt = divmod(ci, 4)
            kw = KTS[kt]
            if kt == 0:
                for hh in range(2):
                    avps[ci, hh] = PS["av"].tile([65, QT], F32, tag="av",
                                                 name="av")
            e = e_tiles[ci, kt]
            for hh in range(2):
                nc.tensor.matmul(
                    avps[ci, hh][:, :],
                    lhsT=vt_sb[p, kt][0:kw, hh * 65:hh * 65 + 65],
                    rhs=e[0:kw, hh * QT:(hh + 1) * QT],
                    start=(kt == 0), stop=(kt == NKT - 1),
                    skip_group_check=True,
                )
            if kt == NKT - 1:
                del e_tiles[ci, kt]

        def emit_drain(ci):
            p, qt = divmod(ci, 4)
            for hh in range(2):
                h = 2 * p + hh
                off = hh * 64
                nc.vector.tensor_copy(
                    attn_sb[p][off:off + 64, qt * QT:(qt + 1) * QT],
                    avps[ci, hh][0:64, :])
                nc.vector.tensor_copy(
                    den_sb[32 * qt:32 * qt + 1, h * QT:(h + 1) * QT],
                    avps[ci, hh][64:65, :])

        def unit_recip(p, hh):
            # fast reciprocal into fp32 scratch, cast bf16 so the broadcast
            # matmul runs at full bf16 rate (fp32 matmuls lower 4x slower)
            h = 2 * p + hh
            nc.vector.reciprocal_approx_fast(
                rscr[:, :], den_sb[:, h * QT:(h + 1) * QT])
            nc.vector.tensor_copy(rec_sb[:, h * QT:(h + 1) * QT], rscr[:, :])

        def unit_bcmul(p, hh, qt):
            # broadcast recip row 32*qt to 64 partitions via basis matmul,
            # then normalize the attn strip. Borrows AV slots so the score
            # stream — the ACT pacer's input — is never blocked.
            h = 2 * p + hh
            off = hh * 64
            bc = PS["av"].tile([64, QT], F32, tag="av", name="av")
            nc.tensor.matmul(
                bc[:, :], lhsT=basis_sb[qt][:, :],
                rhs=rec_sb[:, h * QT:(h + 1) * QT],
                start=True, stop=True)
            nc.vector.tensor_mul(
                attn_sb[p][off:off + 64, qt * QT:(qt + 1) * QT],
                attn_sb[p][off:off + 64, qt * QT:(qt + 1) * QT],
                bc[:, :])

        # prologue: dependency-free dummy matmuls keep the HAM clock-gate warm
        # while the input DMAs land, then pair-0 strip 0 + its vt tiles; the
        # rest of pair 0 is front-loaded filler so the EXP stream starts early
        with tc.tile_pool(name="ps_pro", bufs=6, space="PSUM") as psPro, \
             tc.tile_pool(name="ps_w", bufs=1, space="PSUM") as psW:
            PS["f"] = psPro
            wrm = constP.tile([128, 512], BF16, tag="wrm", name="wrm")
            nc.any.memset(wrm[:, :], 0.0)
            wps = psW.tile([128, 512], F32, tag="w", name="w")
            for i in range(26):
                nc.tensor.matmul(wps[:, :], lhsT=wrm[:, 0:128], rhs=wrm[:, :],
                                 start=(i == 0), stop=(i == 25))
            for u in strip_units(0, 0):
                u()
            for kt in range(3):
                unit_vt(0, kt)

        # steady pipeline pools: scores 2x2 banks, AV 3, filler 1
        with tc.tile_pool(name="ps_s", bufs=2, space="PSUM") as psS, \
             tc.tile_pool(name="ps_av", bufs=3, space="PSUM") as psAV, \
             tc.tile_pool(name="ps_f", bufs=1, space="PSUM") as psF:
            PS["s"], PS["av"], PS["f"] = psS, psAV, psF

            # filler queue: (deadline_step, unit). Pair-0 remnants front-
            # loaded to feed the first chunk's kt progression; pairs 1-5
            # spread over the previous pair's 52-step window.
            filler = []
            strip_dl = {0: 0, 1: 2, 2: 5, 3: 8}
            for qt in (1, 2, 3):
                for u in strip_units(0, qt):
                    filler.append((strip_dl[qt], u))
            for kt in range(3, NKT):
                # vt(0,kt) reads V tokens [128kt, 128kt+kw): must follow the
                # strip that covers its last token, and be ready by g=kt+1
                s_last = (128 * kt + KTS[kt] - 1) // QT
                filler.append((strip_dl[s_last] + 1,
                               lambda kt=kt: unit_vt(0, kt)))
            filler.sort(key=lambda t: t[0])
            kts_by_strip = {}
            for kt in range(NKT):
                kts_by_strip.setdefault((128 * kt + KTS[kt] - 1) // QT,
                                        []).append(kt)
            for p in range(1, 6):
                pu = []
                for qt in range(4):
                    pu.extend(strip_units(p, qt))
                    for kt in kts_by_strip[qt]:
                        pu.append(lambda p=p, kt=kt: unit_vt(p, kt))
                # pair 1's window starts after pair-0 remnants are due
                w0 = 10 if p == 1 else 52 * (p - 1)
                wlen = 52 * p - w0
                npu = len(pu)
                for i, u in enumerate(pu):
                    filler.append((w0 + int((i + 1) * wlen / npu), u))
            # normalization spread one unit per step right after each pair's
            # last drain (which lands at g = 52p + 52)
            for p in range(6):
                gd = 52 * p + 52
                filler.append((gd, lambda p=p: unit_recip(p, 0)))
                filler.append((gd + 1, lambda p=p: unit_recip(p, 1)))
                i = 2
                for hh in range(2):
                    for qt in range(4):
                        filler.append((gd + i,
                                       lambda p=p, hh=hh, qt=qt:
                                       unit_bcmul(p, hh, qt)))
                        i += 1
            filler.sort(key=lambda t: t[0])
            fill_i = 0

            LAG = 1
            for g in range(NCH * NKT + LAG):
                ci, kt = divmod(g, NKT)
                if ci < NCH:
                    emit_scores(ci, kt)
                s = g - LAG
                if s >= 0 and s < NCH * NKT:
                    c2, j = divmod(s, NKT)
                    emit_av(c2, j)
                    if j == NKT - 1:
                        emit_drain(c2)
                while fill_i < len(filler) and filler[fill_i][0] <= g:
                    filler[fill_i][1]()
                    fill_i += 1
            while fill_i < len(filler):
                filler[fill_i][1]()
                fill_i += 1

        # tail: output projection, deep psum pipeline
        with tc.tile_pool(name="ps_t", bufs=6, space="PSUM") as psT:
            PS["f"] = psT
            for m in range(6):
                for qt in range(4):
                    unit_proj(m, qt)

    nc.compile()
    return nc


_CACHE = {}


def _get_nc():
    if "nc" not in _CACHE:
        _CACHE["nc"] = build_nc()
    return _CACHE["nc"]


def make_in_maps(x, qkv_w, pgate_w, pgate_b, proj_w, proj_b):
    bf = ml_dtypes.bfloat16
    x = np.asarray(x, np.float32)
    qkv_w = np.asarray(qkv_w, np.float32)
    pgate_w = np.asarray(pgate_w, np.float32)
    pgate_b = np.asarray(pgate_b, np.float32)
    proj_w = np.asarray(proj_w, np.float32)
    proj_b = np.asarray(proj_b, np.float32)

    common = {
        "qkv_wt": np.ascontiguousarray(qkv_w.T).astype(bf),
        "proj_wt": np.ascontiguousarray(proj_w.T).astype(bf),
        "proj_b": np.ascontiguousarray(proj_b.reshape(C, 1)),
        "ident": np.eye(128, dtype=np.float32).astype(bf),
        # gate bias folded for tanh form: tanh(0.5*pre + 0.5*b)
        "gb": np.concatenate([pgate_b, pgate_b]).reshape(128, 1).astype(np.float32) * 0.5,
    }
    for nm, sl in (("gwq", slice(0, 64)), ("gwk", slice(64, 128)),
                   ("gwv", slice(128, 192))):
        w = pgate_w[:, sl].T  # [d, e] = lhsT
        bd = np.zeros((128, 128), np.float32)
        bd[0:64, 0:64] = w
        bd[64:128, 64:128] = w
        common[nm] = bd.astype(bf)

    return [
        {**common, "xt": np.ascontiguousarray(x[b].T).astype(bf)}
        for b in range(N_CORES)
    ]


def kernel(x, qkv_w, pgate_w, pgate_b, proj_w, proj_b, num_frames=None, **_unused):
    in_maps = make_in_maps(x, qkv_w, pgate_w, pgate_b, proj_w, proj_b)
    nc = _get_nc()
    res = run_bass_kernel_spmd(nc, in_maps, core_ids=list(range(N_CORES)))
    out = np.stack([np.asarray(res.results[b]["out"], np.float32).T
                    for b in range(N_CORES)])
    return np.ascontiguousarray(out)


# revision 37
# speedup vs baseline: 1.0427x; 1.0031x over previous
"""Trainium2 Bass kernel for gated multi-head attention (B=8, N=1568, C=768, H=12).

Sharding: data-parallel over batch — core b computes batch element b entirely
locally (weights replicated), host gathers. Feature-major layouts ([channels,
tokens]) so every matmul contracts on partitions.

Fused single-pipeline design: the attention steady state is paced by ACT's
exp stream (one [kw, 2x392] EXP per (chunk, kt) step, ~0.95us each). All other
work rides in the PE/DVE slack of that stream:

  prologue   pair-0 QKV/gate/vt/zq (the cold-clock HAM warmup IS this real work)
  steady     24 chunks = (pair, qt) x 13 kt steps:
               scores (2 MM, zero-padded K=128 trick) -> EXP -> AV (lag 1 step)
               + interleaved filler: pairs 1-5 QKV chains, gates, V transposes,
                 zq copies; per-pair normalization (denominator recip +
                 basis-broadcast matmuls)
  tail       output projection (24 chains) + DMA out

PSUM budget: score slots 2x[128,2,512] (4 banks) + AV accumulators bufs=3
(2 active + 1 draining, [65,392] each) + 1 filler bank (QKV/gate/transpose/
proj chains, one at a time) = 8 banks.

Gate: sigmoid via tanh (same ACT table set as exp); Qg' = (tanh+1)*Q =
2*sigmoid*Q, the 2x per side folded into the exp scale (SCALE/4). Softmax
needs no max-subtraction: |scores*scale| < 1 for this data. Denominator rides
in AV row 64 via a ones-column in vt; normalization multiplies by a
basis-matmul broadcast of the reciprocal.
"""

import numpy as np
import ml_dtypes
from contextlib import ExitStack

import concourse.bass as bass
import concourse.tile as tile
from concourse import bacc, mybir
from concourse.bass_utils import run_bass_kernel_spmd

BF16 = mybir.dt.bfloat16
F32 = mybir.dt.float32
AF = mybir.ActivationFunctionType
ALU = mybir.AluOpType

N_CORES = 8
N, C, H, HD = 1568, 768, 12, 64
SCALE = HD ** -0.5
QT = 392            # token free-dim tile (4 per pair)
KTS = [128] * 12 + [32]   # token partition tiles
NKT = len(KTS)
NCH = 24            # chunks = (pair, qt)


def build_nc():
    nc = bacc.Bacc(
        "TRN2",
        target_bir_lowering=False,
        debug=False,
        enable_asserts=False,
        num_devices=N_CORES,
    )
    xt = nc.dram_tensor("xt", [C, N], BF16, kind="ExternalInput").ap()
    qkv_wt = nc.dram_tensor("qkv_wt", [C, 3 * C], BF16, kind="ExternalInput").ap()
    gwq = nc.dram_tensor("gwq", [128, 128], BF16, kind="ExternalInput").ap()
    gwk = nc.dram_tensor("gwk", [128, 128], BF16, kind="ExternalInput").ap()
    gwv = nc.dram_tensor("gwv", [128, 128], BF16, kind="ExternalInput").ap()
    gb = nc.dram_tensor("gb", [128, 1], F32, kind="ExternalInput").ap()
    proj_wt = nc.dram_tensor("proj_wt", [C, C], BF16, kind="ExternalInput").ap()
    proj_b = nc.dram_tensor("proj_b", [C, 1], F32, kind="ExternalInput").ap()
    ident = nc.dram_tensor("ident", [128, 128], BF16, kind="ExternalInput").ap()
    out = nc.dram_tensor("out", [C, N], F32, kind="ExternalOutput").ap()

    with tile.TileContext(nc) as tc, ExitStack() as ES:
        constP = ES.enter_context(tc.tile_pool(name="const", bufs=1))
        kgP = ES.enter_context(tc.tile_pool(name="kgsb", bufs=1))
        qvP = ES.enter_context(tc.tile_pool(name="qvsb", bufs=3))
        vtP = ES.enter_context(tc.tile_pool(name="vtsb", bufs=1))
        attnP = ES.enter_context(tc.tile_pool(name="attnsb", bufs=1))
        zqP = ES.enter_context(tc.tile_pool(name="zqp", bufs=1))
        eP = ES.enter_context(tc.tile_pool(name="esb", bufs=5))
        oP = ES.enter_context(tc.tile_pool(name="osb", bufs=4))
        # PSUM pools are opened in phase scopes below; units reach the current
        # filler pool via PS["f"].
        PS = {}

        # ---------------- DMA in (priority order) ----------------
        xt_sb = []
        qw_sb = []
        for k in range(6):
            xt_sb.append(constP.tile([128, N], BF16, tag=f"xt{k}", name=f"xt{k}"))
            nc.sync.dma_start(xt_sb[k][:, :], xt[k * 128:(k + 1) * 128, :])
            qw_sb.append(constP.tile([128, 3 * C], BF16, tag=f"qw{k}", name=f"qw{k}"))
            nc.sync.dma_start(qw_sb[k][:, :], qkv_wt[k * 128:(k + 1) * 128, :])
        ident_sb = constP.tile([128, 128], BF16, tag="ident", name="ident")
        nc.sync.dma_start(ident_sb[:, :], ident)
        gw_sb = {}
        for nm, t in (("gwq", gwq), ("gwk", gwk), ("gwv", gwv)):
            gw_sb[nm] = constP.tile([128, 128], BF16, tag=nm, name=nm)
            nc.sync.dma_start(gw_sb[nm][:, :], t)
        gb_sb = constP.tile([128, 1], F32, tag="gb", name="gb")
        nc.sync.dma_start(gb_sb[:, :], gb)
        pw_sb = []
        pb_sb = []
        for k in range(6):
            pw_sb.append(constP.tile([128, C], BF16, tag=f"pw{k}", name=f"pw{k}"))
            nc.sync.dma_start(pw_sb[k][:, :], proj_wt[k * 128:(k + 1) * 128, :])
            pb_sb.append(constP.tile([128, 1], F32, tag=f"pb{k}", name=f"pb{k}"))
            nc.sync.dma_start(pb_sb[k][:, :], proj_b[k * 128:(k + 1) * 128, :])

        # ---------------- static SBUF ----------------
        # basis[qt]: one-hot row 32*qt for the reciprocal broadcast matmul
        basis_sb = []
        for qt in range(4):
            bt = constP.tile([128, 64], BF16, tag=f"basis{qt}", name=f"basis{qt}")
            nc.any.memset(bt[:, :], 0.0)
            nc.any.memset(bt[32 * qt:32 * qt + 1, :], 1.0)
            basis_sb.append(bt)

        kg_sb = [kgP.tile([128, N], BF16, tag=f"kg{p}", name=f"kg{p}")
                 for p in range(6)]
        attn_sb = [attnP.tile([128, N], BF16, tag=f"a{p}", name=f"a{p}")
                   for p in range(6)]
        den_sb = attnP.tile([128, H * QT], F32, tag="den", name="den")
        rec_sb = attnP.tile([128, H * QT], BF16, tag="recip", name="recip")
        rscr = attnP.tile([128, QT], F32, tag="rscr", name="rscr")
        nc.any.memset(den_sb[:, :], 1.0)
        nc.any.memset(rec_sb[:, :], 0.0)

        # per-(pair, kt) V^T tiles: [kw, 2 heads x (64 d + ones-col)] — separate
        # tiles per pair so pair p+1's transposes have no WAR on pair p's AV
        # reads
        vt_sb = {}
        for p in range(6):
            for kt in range(NKT):
                t = vtP.tile([128, 130], BF16, tag=f"vt{p}_{kt}",
                             name=f"vt{p}_{kt}")
                ones_col = t[:, :].rearrange("p (h e) -> p h e", e=65)[:, :, 64]
                nc.any.memset(ones_col, 1.0)
                vt_sb[p, kt] = t

        # zero-padded per-head Q, two alternating persistent sets
        zq_sb = {}
        for s in range(2):
            for hh in range(2):
                off = hh * 64
                z = zqP.tile([128, N], BF16, tag=f"zq{s}{hh}", name=f"zq{s}{hh}")
                nc.gpsimd.memset(z[64 - off:128 - off, :], 0.0)
                zq_sb[s, hh] = z

        # ---------------- filler work units ----------------
        # Each unit emits a small PE chain (+ its drain) through the single
        # filler PSUM bank. Units are generators of nothing; emission order is
        # the schedule.
        q_tiles = {}
        v_tiles = {}

        def unit_qkv(p, which, qt):
            # which: 0=Q(m=p),1=K(m=6+p),2=V(m=12+p); one qt strip
            m = p + 6 * which
            sl = slice(qt * QT, (qt + 1) * QT)
            if which == 0:
                if p not in q_tiles:
                    q_tiles[p] = qvP.tile([128, N], BF16, tag="q", name=f"q{p}")
                dst = q_tiles[p]
            elif which == 1:
                dst = kg_sb[p]
            else:
                if p not in v_tiles:
                    v_tiles[p] = qvP.tile([128, N], BF16, tag="v", name=f"v{p}")
                dst = v_tiles[p]
            ps = PS["f"].tile([128, QT], F32, tag="f", name="f")
            for k in range(6):
                nc.tensor.matmul(ps[:, :],
                                 lhsT=qw_sb[k][:, m * 128:(m + 1) * 128],
                                 rhs=xt_sb[k][:, sl],
                                 start=(k == 0), stop=(k == 5))
            nc.vector.tensor_copy(dst[:, sl], ps[:, :])

        def unit_gate(p, qt):
            sl = slice(qt * QT, (qt + 1) * QT)
            g = eP.tile([128, QT], BF16, tag="g", name="g")
            ps = PS["f"].tile([128, QT], F32, tag="f", name="f")
            nc.tensor.matmul(ps[:, :], lhsT=gw_sb["gwq"][:, :],
                             rhs=q_tiles[p][:, sl], start=True, stop=False)
            nc.tensor.matmul(ps[:, :], lhsT=gw_sb["gwk"][:, :],
                             rhs=kg_sb[p][:, sl], start=False, stop=False)
            nc.tensor.matmul(ps[:, :], lhsT=gw_sb["gwv"][:, :],
                             rhs=v_tiles[p][:, sl], start=False, stop=True)
            nc.scalar.activation(g[:, :], ps[:, :], AF.Tanh,
                                 bias=gb_sb[:, 0:1], scale=0.5)
            # Qg' = (t+1)*Q in place; Kg' = (t+1)*K in place
            nc.vector.scalar_tensor_tensor(
                q_tiles[p][:, sl], g[:, :], 1.0, q_tiles[p][:, sl],
                op0=ALU.add, op1=ALU.mult)
            nc.vector.scalar_tensor_tensor(
                kg_sb[p][:, sl], g[:, :], 1.0, kg_sb[p][:, sl],
                op0=ALU.add, op1=ALU.mult)

        def unit_zq(p, hh, qt):
            off = hh * 64
            sl = slice(qt * QT, (qt + 1) * QT)
            nc.vector.tensor_copy(zq_sb[p % 2, hh][off:off + 64, sl],
                                  q_tiles[p][off:off + 64, sl])

        def unit_vt(p, kt):
            kw = KTS[kt]
            vsrc = v_tiles[p][:, kt * 128:kt * 128 + kw]
            ps = PS["f"].tile([128, 128], BF16, tag="f", name="f")
            nc.tensor.transpose(ps[0:kw, 0:128], vsrc, ident_sb[:, :])
            dst = vt_sb[p, kt][0:kw, :].rearrange(
                "p (h e) -> p h e", e=65)[:, :, 0:64]
            nc.vector.tensor_copy(
                dst, ps[0:kw, :].rearrange("p (h e) -> p h e", e=64))

        def unit_proj(m, qt):
            sl = slice(qt * QT, (qt + 1) * QT)
            ps = PS["f"].tile([128, QT], F32, tag="f", name="f")
            for k in range(6):
                nc.tensor.matmul(ps[:, :],
                                 lhsT=pw_sb[k][:, m * 128:(m + 1) * 128],
                                 rhs=attn_sb[k][:, sl],
                                 start=(k == 0), stop=(k == 5))
            o = oP.tile([128, QT], F32, tag="o", name="o")
            nc.scalar.activation(o[:, :], ps[:, :], AF.Identity,
                                 bias=pb_sb[m][:, 0:1])
            nc.sync.dma_start(out[m * 128:(m + 1) * 128, sl], o[:, :])

        def strip_units(p, qt):
            # all prep for one qt strip of a pair: K,V,Q halves + gate + zq
            u = []
            for which in (1, 2, 0):   # K first (scores lhsT), V, Q
                u.append(lambda p=p, w=which, qt=qt: unit_qkv(p, w, qt))
            u.append(lambda p=p, qt=qt: unit_gate(p, qt))
            u.append(lambda p=p, qt=qt: unit_zq(p, 0, qt))
            u.append(lambda p=p, qt=qt: unit_zq(p, 1, qt))
            return u

        # ---------------- attention pipeline ----------------
        e_tiles = {}
        avps = {}

        def emit_scores(ci, kt):
            p, qt = divmod(ci, 4)
            kw = KTS[kt]
            kb = kt * 128
            sps = PS["s"].tile([128, 2, 512], F32, tag="s", name="s")
            for hh in range(2):
                nc.tensor.matmul(
                    sps[0:kw, hh, 0:QT],
                    lhsT=kg_sb[p][:, kb:kb + kw],
                    rhs=zq_sb[p % 2, hh][:, qt * QT:(qt + 1) * QT],
                    start=True, stop=True,
                )
            e = eP.tile([128, 2 * QT], BF16, tag="e", name="e")
            nc.scalar.activation(
                e[0:kw, :].rearrange("p (s n) -> p s n", s=2),
                sps[0:kw, :, 0:QT], AF.Exp, scale=SCALE / 4.0,
            )
            e_tiles[ci, kt] = e

        def emit_av(ci, kt):
            p, qt = divmod(ci, 4)
            kw = KTS[kt]
            if kt == 0:
                for hh in range(2):
                    avps[ci, hh] = PS["av"].tile([65, QT], F32, tag="av",
                                                 name="av")
            e = e_tiles[ci, kt]
            for hh in range(2):
                nc.tensor.matmul(
                    avps[ci, hh][:, :],
                    lhsT=vt_sb[p, kt][0:kw, hh * 65:hh * 65 + 65],
                    rhs=e[0:kw, hh * QT:(hh + 1) * QT],
                    start=(kt == 0), stop=(kt == NKT - 1),
                    skip_group_check=True,
                )
            if kt == NKT - 1:
                del e_tiles[ci, kt]

        def emit_drain(ci):
            p, qt = divmod(ci, 4)
            for hh in range(2):
                h = 2 * p + hh
                off = hh * 64
                nc.vector.tensor_copy(
                    attn_sb[p][off:off + 64, qt * QT:(qt + 1) * QT],
                    avps[ci, hh][0:64, :])
                nc.vector.tensor_copy(
                    den_sb[32 * qt:32 * qt + 1, h * QT:(h + 1) * QT],
                    avps[ci, hh][64:65, :])

        def unit_norm(p, hh, qt):
            # per-strip normalization right after the chunk's drain: fast
            # reciprocal (free-dim-driven cost, so per-strip is no more
            # expensive), bf16 cast (fp32 matmuls lower 4x slower), basis
            # broadcast into an AV slot (never blocks the score stream),
            # multiply. Rows of other strips recompute idempotently.
            h = 2 * p + hh
            off = hh * 64
            seg = slice(h * QT, (h + 1) * QT)
            nc.vector.reciprocal_approx_fast(rscr[:, :], den_sb[:, seg])
            nc.vector.tensor_copy(rec_sb[:, seg], rscr[:, :])
            bc = PS["f"].tile([64, QT], F32, tag="f", name="f")
            nc.tensor.matmul(
                bc[:, :], lhsT=basis_sb[qt][:, :],
                rhs=rec_sb[:, seg], start=True, stop=True)
            nc.vector.tensor_mul(
                attn_sb[p][off:off + 64, qt * QT:(qt + 1) * QT],
                attn_sb[p][off:off + 64, qt * QT:(qt + 1) * QT],
                bc[:, :])

        # prologue: dependency-free dummy matmuls keep the HAM clock-gate warm
        # while the input DMAs land, then pair-0 strip 0 + its vt tiles; the
        # rest of pair 0 is front-loaded filler so the EXP stream starts early
        with tc.tile_pool(name="ps_pro", bufs=6, space="PSUM") as psPro, \
             tc.tile_pool(name="ps_w", bufs=1, space="PSUM") as psW:
            PS["f"] = psPro
            wrm = constP.tile([128, 512], BF16, tag="wrm", name="wrm")
            nc.any.memset(wrm[:, :], 0.0)
            wps = psW.tile([128, 512], F32, tag="w", name="w")
            for i in range(26):
                nc.tensor.matmul(wps[:, :], lhsT=wrm[:, 0:128], rhs=wrm[:, :],
                                 start=(i == 0), stop=(i == 25))
            for u in strip_units(0, 0):
                u()
            for kt in range(3):
                unit_vt(0, kt)

        # steady pipeline pools: scores 2x2 banks, AV 3, filler 1
        with tc.tile_pool(name="ps_s", bufs=2, space="PSUM") as psS, \
             tc.tile_pool(name="ps_av", bufs=3, space="PSUM") as psAV, \
             tc.tile_pool(name="ps_f", bufs=1, space="PSUM") as psF:
            PS["s"], PS["av"], PS["f"] = psS, psAV, psF

            # filler queue: (deadline_step, unit). Pair-0 remnants front-
            # loaded to feed the first chunk's kt progression; pairs 1-5
            # spread over the previous pair's 52-step window.
            filler = []
            strip_dl = {0: 0, 1: 2, 2: 5, 3: 8}
            for qt in (1, 2, 3):
                for u in strip_units(0, qt):
                    filler.append((strip_dl[qt], u))
            for kt in range(3, NKT):
                # vt(0,kt) reads V tokens [128kt, 128kt+kw): must follow the
                # strip that covers its last token, and be ready by g=kt+1
                s_last = (128 * kt + KTS[kt] - 1) // QT
                filler.append((strip_dl[s_last] + 1,
                               lambda kt=kt: unit_vt(0, kt)))
            filler.sort(key=lambda t: t[0])
            kts_by_strip = {}
            for kt in range(NKT):
                kts_by_strip.setdefault((128 * kt + KTS[kt] - 1) // QT,
                                        []).append(kt)
            for p in range(1, 6):
                pu = []
                for qt in range(4):
                    pu.extend(strip_units(p, qt))
                    for kt in kts_by_strip[qt]:
                        pu.append(lambda p=p, kt=kt: unit_vt(p, kt))
                # pair 1's window starts after pair-0 remnants are due
                w0 = 10 if p == 1 else 52 * (p - 1)
                wlen = 52 * p - w0
                npu = len(pu)
                for i, u in enumerate(pu):
                    filler.append((w0 + int((i + 1) * wlen / npu), u))
            # per-strip normalization right after each chunk's drain (which
            # lands at g = 13*ci + 13), staggered so DVE isn't spiked
            for ci in range(NCH):
                p, qt = divmod(ci, 4)
                gd = 13 * ci + 13
                filler.append((gd + 1, lambda p=p, qt=qt: unit_norm(p, 0, qt)))
                filler.append((gd + 4, lambda p=p, qt=qt: unit_norm(p, 1, qt)))
            # projection per qt strip as soon as pair 5's strip is normed;
            # qt<3 overlaps the last chunks, qt=3 flushes after the loop
            for qt in range(4):
                for m in range(6):
                    filler.append((13 * (20 + qt) + 18 + m,
                                   lambda m=m, qt=qt: unit_proj(m, qt)))
            filler.sort(key=lambda t: t[0])
            fill_i = 0

            LAG = 1
            for g in range(NCH * NKT + LAG):
                ci, kt = divmod(g, NKT)
                if ci < NCH:
                    emit_scores(ci, kt)
                s = g - LAG
                if s >= 0 and s < NCH * NKT:
                    c2, j = divmod(s, NKT)
                    emit_av(c2, j)
                    if j == NKT - 1:
                        emit_drain(c2)
                while fill_i < len(filler) and filler[fill_i][0] <= g:
                    filler[fill_i][1]()
                    fill_i += 1
            while fill_i < len(filler):
                filler[fill_i][1]()
                fill_i += 1

    nc.compile()
    return nc


_CACHE = {}


def _get_nc():
    if "nc" not in _CACHE:
        _CACHE["nc"] = build_nc()
    return _CACHE["nc"]


def make_in_maps(x, qkv_w, pgate_w, pgate_b, proj_w, proj_b):
    bf = ml_dtypes.bfloat16
    x = np.asarray(x, np.float32)
    qkv_w = np.asarray(qkv_w, np.float32)
    pgate_w = np.asarray(pgate_w, np.float32)
    pgate_b = np.asarray(pgate_b, np.float32)
    proj_w = np.asarray(proj_w, np.float32)
    proj_b = np.asarray(proj_b, np.float32)

    common = {
        "qkv_wt": np.ascontiguousarray(qkv_w.T).astype(bf),
        "proj_wt": np.ascontiguousarray(proj_w.T).astype(bf),
        "proj_b": np.ascontiguousarray(proj_b.reshape(C, 1)),
        "ident": np.eye(128, dtype=np.float32).astype(bf),
        # gate bias folded for tanh form: tanh(0.5*pre + 0.5*b)
        "gb": np.concatenate([pgate_b, pgate_b]).reshape(128, 1).astype(np.float32) * 0.5,
    }
    for nm, sl in (("gwq", slice(0, 64)), ("gwk", slice(64, 128)),
                   ("gwv", slice(128, 192))):
        w = pgate_w[:, sl].T  # [d, e] = lhsT
        bd = np.zeros((128, 128), np.float32)
        bd[0:64, 0:64] = w
        bd[64:128, 64:128] = w
        common[nm] = bd.astype(bf)

    return [
        {**common, "xt": np.ascontiguousarray(x[b].T).astype(bf)}
        for b in range(N_CORES)
    ]


def kernel(x, qkv_w, pgate_w, pgate_b, proj_w, proj_b, num_frames=None, **_unused):
    in_maps = make_in_maps(x, qkv_w, pgate_w, pgate_b, proj_w, proj_b)
    nc = _get_nc()
    res = run_bass_kernel_spmd(nc, in_maps, core_ids=list(range(N_CORES)))
    out = np.stack([np.asarray(res.results[b]["out"], np.float32).T
                    for b in range(N_CORES)])
    return np.ascontiguousarray(out)
